# revision 63
# baseline (speedup 1.0000x reference)
"""Trainium2 Bass kernel for nn_KernelFilter_S (dynamic per-sample filter CNN).

Data-parallel over batch B=8 across 8 NeuronCores (one sample per core).

Per-core math (sample x = content[b], s = style[b]):
  c1 = conv3x3(x, ds_w) + ds_b                       [32,64,64]
  pooled_F = mean_HW(conv3x3(s, cwF)) + cbF          [32]    (F = 1,2)
  filtF = (pooled_F @ fwF.T + fbF).reshape(32,32,3,3)
  c2 = leaky(conv3x3_dyn(c1, filt1), 0.2)
  c3 = conv3x3_dyn(c2, filt2)
  out = x + conv3x3(c3, up_w) + up_b                 [512,64,64]

Implementation notes (v2):
  * fp8e4 + DoubleRow matmuls for all four image convs (2 k-tiles of up to
    128 partitions per matmul, selected via an explicit strided dim-1 in the
    access pattern - so the two ky tap rows of a conv come from the SAME
    image at col offsets differing by 66, no data duplication).
  * ds conv M-packs kx: psum rows are (kx, o); combined with +-1 column
    shifts by two vector adds + one activation (window widened by 2 cols so
    the combine stays tile-local).
  * dyn convs read a kx-stacked image c*stack[(kx,i), :] (center written by
    the producing conv's activation; kx=0/2 blocks are SBUF->SBUF DMA copies
    at shifted columns), taps over ky via DoubleRow dim-1 stride 66.
  * up conv: contraction (kx,i)=96 over c3stack + ky via DoubleRow; up_b is
    folded in as a 97th constant partition row.
  * mean-pool-of-conv for the filter predictors needs only 9 rectangle sums
    R[i,t] per style channel (computed on DVE); the 32->9216 FC runs as 48
    matmuls of N=4 using a block-diagonal pooled operand.
  * content arrives host-prepadded in fp8 (guard ring baked), plus bf16 for
    the residual; output returned bf16 and upcast on host.
"""

import os
import sys
import numpy as np

sys.path.insert(0, "/opt/trn_rl_repo")

import concourse.bass as bass
import concourse.bacc as bacc
import concourse.mybir as mybir
import concourse.tile as tile
from concourse.bass_utils import run_bass_kernel_spmd

F32 = mybir.dt.float32
BF16 = mybir.dt.bfloat16
FP8 = mybir.dt.float8e4
NP_BF16 = np.dtype(mybir.dt.np(BF16))
NP_FP8 = np.dtype(mybir.dt.np(FP8))

H = W = 64
PW = W + 2              # padded row width = 66
NPIX = H * W            # 4096
NPAD = (H + 2) * PW     # 66*66 = 4356
GUARD = PW + 1          # 67
BUFW = GUARD + NPAD + GUARD  # 4490
CIN = 512
INNER = 32

# scale factors (compensated at psum->sbuf writes)
S_DSW = 8.0     # ds_w prescale        -> ds psum = 8*c1
S_F = 128.0     # filt prescale (via fw/fb) -> filt' = 128*filt
S_C2 = 16.0     # c2 stored as 16*c2   (dyn1 write scale 16/128 = 1/8)
S_C3 = 256.0    # c3 stored as 256*c3  (dyn2 write scale 256/(16*128) = 1/8)
S_UPW = 32.0    # up_w prescale        -> up psum = 32*256*delta = 8192*delta
BIAS_C0 = 128.0  # constant value of the c3stack bias row

ROW_TILES = [(r0, 7) for r0 in range(0, 63, 7)] + [(63, 1)]

Identity = mybir.ActivationFunctionType.Identity
Lrelu = mybir.ActivationFunctionType.Lrelu
AluAdd = mybir.AluOpType.add
AluMult = mybir.AluOpType.mult
DR = mybir.MatmulPerfMode.DoubleRow


def _interior(ap, nr):
    return ap.rearrange("p (r x) -> p r x", x=PW)[:, :, 1:1 + W]


REPL_WINDOWS = {21: (0, 28), 42: (28, 49), 63: (49, 64)}


def _replicate(nc, stack, r0_last):
    """Copy the kx=0/2 partition blocks of a stack from the center block,
    shifted by +-1 column, for the row window ending at tile r0_last.
    Contiguous full-row spans: pad cols in the source are zero, and they land
    on pad positions in the destination, so the copy stays consistent."""
    rs, re = REPL_WINDOWS[r0_last]
    cs = GUARD + (rs + 1) * PW
    ce = GUARD + (re + 1) * PW
    src = stack[32:64, cs:ce]
    nc.sync.dma_start(out=stack[0:32, cs + 1:ce + 1], in_=src)
    nc.sync.dma_start(out=stack[64:96, cs - 1:ce - 1], in_=src)


def _ky_pair_ap(stack_ap, base, n):
    """[96, 2, n] view of a [96+, BUFW] stack: dim1 = ky in {0,1}, stride PW."""
    a = stack_ap[0:96, base:base + n]
    pairs = [list(p) for p in a.ap]
    new = [pairs[0], [PW, 2], pairs[-1]]
    return bass.AP(a.tensor, a.offset, new)


def _chunk_pair_ap(pair_ap, base, n):
    """[128, 2, n] view of a [128, 2*BUFW] chunk-pair tile: dim1 = chunk."""
    a = pair_ap[:, base:base + n]
    pairs = [list(p) for p in a.ap]
    new = [pairs[0], [BUFW, 2], pairs[-1]]
    return bass.AP(a.tensor, a.offset, new)


def _build_program():
    nc = bacc.Bacc(None, target_bir_lowering=False)

    cpad_h = [nc.dram_tensor(f"cpad{p}", [128, 2 * BUFW], FP8, kind="ExternalInput")
              for p in range(2)]
    wds_h = nc.dram_tensor("w_ds", [128, 6 * 192], FP8, kind="ExternalInput")
    wcw_h = nc.dram_tensor("w_cw", [128, 36 * 64], BF16, kind="ExternalInput")
    styleT_h = nc.dram_tensor("styleT", [128, 32 * CIN], FP8, kind="ExternalInput")
    mask_h = nc.dram_tensor("mask", [128, 32 * 16], FP8, kind="ExternalInput")
    eye_h = nc.dram_tensor("eye16", [16, 16], BF16, kind="ExternalInput")
    wfc_h = nc.dram_tensor("w_fc", [128, 48 * 96], BF16, kind="ExternalInput")
    fbl_h = nc.dram_tensor("fb_l", [96, 192], F32, kind="ExternalInput")
    dsb_h = nc.dram_tensor("ds_b", [INNER], F32, kind="ExternalInput")
    cbb_h = nc.dram_tensor("cb_b", [64], F32, kind="ExternalInput")
    wup_h = nc.dram_tensor("w_up", [97, 12 * 128], FP8, kind="ExternalInput")
    cont_h = nc.dram_tensor("content_bf", [CIN, NPIX], BF16, kind="ExternalInput")
    out_h = nc.dram_tensor("out", [CIN, NPIX], BF16, kind="ExternalOutput")

    with tile.TileContext(nc) as tc:
        with (
            tc.tile_pool(name="const", bufs=1) as const,
            tc.tile_pool(name="big", bufs=1) as big,
            tc.tile_pool(name="work", bufs=4) as work,
            tc.tile_pool(name="ds_ps", bufs=3, space=bass.MemorySpace.PSUM) as ds_psp,
            tc.tile_pool(name="dyn_ps", bufs=2, space=bass.MemorySpace.PSUM) as dyn_psp,
            tc.tile_pool(name="up_ps", bufs=2, space=bass.MemorySpace.PSUM) as up_psp,
            tc.tile_pool(name="pred_ps", bufs=1, space=bass.MemorySpace.PSUM) as pred_psp,
        ):
            # ---- DMA loads (issue order = HBM service order) -------------
            mask_sb = const.tile([128, 32 * 16], FP8, tag="mask")
            nc.sync.dma_start(out=mask_sb[:], in_=mask_h[:])
            eye_sb = const.tile([16, 16], BF16, tag="eye")
            nc.sync.dma_start(out=eye_sb[:], in_=eye_h[:])
            styT_sb = big.tile([128, 32 * CIN], FP8, tag="styleT")
            nc.sync.dma_start(out=styT_sb[:, 0:16 * CIN],
                              in_=styleT_h[:][:, 0:16 * CIN])
            nc.sync.dma_start(out=styT_sb[:, 16 * CIN:],
                              in_=styleT_h[:][:, 16 * CIN:])
            cpad = []
            for p in range(2):
                t = big.tile([128, 2 * BUFW], FP8, tag=f"cpad{p}")
                nc.sync.dma_start(out=t[:], in_=cpad_h[p][:])
                cpad.append(t)
            wds_sb = const.tile([128, 6 * 192], FP8, tag="wds")
            nc.sync.dma_start(out=wds_sb[:], in_=wds_h[:])
            wcw_sb = const.tile([128, 36 * 64], BF16, tag="wcw")
            wfc_sb = const.tile([128, 48 * 96], BF16, tag="wfc")
            fbl_sb = const.tile([96, 192], F32, tag="fbl")
            nc.sync.dma_start(out=fbl_sb[:], in_=fbl_h[:])
            dsb_sb = const.tile([INNER, 1], F32, tag="dsb")
            nc.sync.dma_start(out=dsb_sb[:], in_=dsb_h[:].rearrange("(o u) -> o u", u=1))
            cbb_sb = const.tile([64, 1], F32, tag="cbb")
            nc.sync.dma_start(out=cbb_sb[:], in_=cbb_h[:].rearrange("(o u) -> o u", u=1))

            # ---- stacked image buffers: zero only pads/guards ------------
            c1stack = big.tile([96, BUFW], FP8, tag="c1stack")
            c2stack = big.tile([96, BUFW], FP8, tag="c2stack")
            c3stack = big.tile([97, BUFW], FP8, tag="c3stack")
            for stk in (c1stack, c2stack, c3stack):
                # guards + padded row 0 / row 65 + per-row pad col pairs
                nc.vector.memset(stk[0:96, 0:GUARD + PW + 1], 0.0)
                nc.vector.memset(stk[0:96, GUARD + 65 * PW:BUFW], 0.0)
                nc.vector.memset(
                    stk[0:96, GUARD + PW + 65:GUARD + PW + 65 + 64 * PW]
                    .rearrange("p (r x) -> p r x", x=PW)[:, :, 0:2], 0.0)
            nc.gpsimd.memset(c3stack[96:97, :], BIAS_C0)

            Rcb_all = big.tile([128, 64], BF16, tag="rcb")

            wds_v = wds_sb[:].rearrange("p (e j m) -> p e j m", e=6, j=2)
            wcw_v = wcw_sb[:].rearrange("p (c t m) -> p c t m", c=4, t=9)
            wfc_v = wfc_sb[:].rearrange("p (f g m) -> p f g m", f=2, g=24)

            def defer_load(tile_, dram_ap):
                # Order the load behind styleT's arrival: a marker op reads
                # styT (RAW on its DMA) and writes the target tile (WAW with
                # the load), so deferred loads don't compete for HBM early.
                nc.gpsimd.tensor_copy(tile_[0:1, 0:1], wds_sb[0:1, 0:1])
                nc.sync.dma_start(out=tile_[:], in_=dram_ap)

            # ---- ds conv: content -> c1stack -----------------------------
            def ds_tile(r0, nr):
                N = nr * PW
                N2 = N + 2
                col0 = GUARD + (r0 + 1) * PW
                pst = ds_psp.tile([96, 7 * PW + 2], F32, tag="ds_ps")
                ps = pst[:, 0:N2]
                k = 0
                for p in range(2):
                    for ky in range(3):
                        nc.tensor.matmul(
                            ps,
                            wds_v[:, ky * 2 + p, :, :],
                            _chunk_pair_ap(cpad[p][:], col0 - 1 + (ky - 1) * PW, N2),
                            start=(k == 0), stop=(k == 5), perf_mode=DR,
                        )
                        k += 1
                # combine kx blocks: c1[o,C] = sum_kx ps[(kx,o), C+kx-1]
                # (engines read at most one PSUM operand per instruction;
                #  scale/bias folded into the ACT + STT chain)
                t1t = work.tile([INNER, 7 * PW], F32, tag="dscomb")
                t1 = t1t[:, 0:N]
                nc.scalar.activation(t1, pst[0:32, 0:N], Identity,
                                     bias=dsb_sb[:], scale=1.0 / S_DSW)
                nc.vector.scalar_tensor_tensor(
                    t1, pst[32:64, 1:N + 1], 1.0 / S_DSW, t1,
                    op0=AluMult, op1=AluAdd)
                nc.vector.scalar_tensor_tensor(
                    _interior(c1stack[32:64, col0:col0 + N], nr),
                    _interior(pst[64:96, 2:N + 2], nr),
                    1.0 / S_DSW,
                    _interior(t1, nr),
                    op0=AluMult, op1=AluAdd)
                if r0 in (21, 42, 63):
                    _replicate(nc, c1stack, r0)

            defer_load(wcw_sb, wcw_h[:])
            defer_load(wfc_sb, wfc_h[:])

            # ---- style rectangle sums on the PE: R^T = mask^T @ styleT ---
            # (9 rectangle masks (padded to 16) x 512 channels, contraction
            #  over pixels; transposed back to [ch, (c,t)] by PE transposes)
            mask_v = mask_sb[:].rearrange("p (k t) -> p k t", k=32)
            styT_v = styT_sb[:].rearrange("p (k c) -> p k c", k=32)
            rt_full = pred_psp.tile([128, CIN], F32, tag="pred")
            rt_ps = rt_full[0:16, :]
            for k in range(16):
                nc.tensor.matmul(rt_ps[:],
                                 mask_v[:, 2 * k:2 * k + 2, :],
                                 styT_v[:, 2 * k:2 * k + 2, :],
                                 start=(k == 0), stop=(k == 15), perf_mode=DR)
            rt_sb = work.tile([16, CIN], BF16, tag="rt_sb")
            nc.scalar.activation(rt_sb[:], rt_ps[:],
                                 mybir.ActivationFunctionType.Copy)
            rtt = pred_psp.tile([128, CIN], F32, tag="pred")
            rtt_bf = rtt[:, 0:32].bitcast(BF16)
            for c in range(4):
                nc.tensor.transpose(rtt_bf[:, c * 16:(c + 1) * 16],
                                    rt_sb[:, c * 128:(c + 1) * 128],
                                    eye_sb[:])
            nc.scalar.activation(Rcb_all[:], rtt_bf,
                                 mybir.ActivationFunctionType.Copy)

            # ---- filter predictor ---------------------------------------
            pred0 = pred_psp.tile([128, CIN], F32, tag="pred")
            pool_ps = pred0[0:64, 96:97]
            k = 0
            for t in range(9):
                for c in range(4):
                    nc.tensor.matmul(
                        pool_ps, wcw_v[:, c, t, :], Rcb_all[:, c * 16 + t:c * 16 + t + 1],
                        start=(k == 0), stop=(k == 35))
                    k += 1
            pooled = work.tile([64, 1], BF16, tag="pooled")
            nc.scalar.activation(pooled[:], pool_ps, Identity,
                                 bias=cbb_sb[:], scale=1.0 / NPIX)
            pdiag = const.tile([128, 8], BF16, tag="pdiag")
            nc.gpsimd.memset(pdiag[:], 0.0)
            for F in range(2):
                for g4 in range(4):
                    nc.gpsimd.tensor_copy(
                        pdiag[g4 * 32:(g4 + 1) * 32, F * 4 + g4:F * 4 + g4 + 1],
                        pooled[F * 32:(F + 1) * 32, :])
            filt = []
            for F in range(2):
                fpt = pred_psp.tile([128, CIN], F32, tag="pred")
                fps = fpt[0:96, 0:96]
                for g in range(24):
                    nc.tensor.matmul(
                        fps[:, g * 4:(g + 1) * 4],
                        wfc_v[:, F, g, :],
                        pdiag[:, F * 4:(F + 1) * 4],
                        start=True, stop=True)
                ft = const.tile([96, 96], FP8, tag=f"filt{F}")
                nc.vector.tensor_add(ft[:], fps, fbl_sb[:, F * 96:(F + 1) * 96])
                filt.append(ft)

            # ---- ds conv body ----
            for (r0, nr) in ROW_TILES:
                ds_tile(r0, nr)

            # deferred loads: needed only from the up-conv phase onward
            wup_sb = const.tile([97, 12 * 128], FP8, tag="wup")
            defer_load(wup_sb, wup_h[:])
            cont = []
            for c in range(4):
                t = big.tile([128, NPIX], BF16, tag=f"cont{c}")
                defer_load(t, cont_h[:].rearrange("(c p) q -> c p q", p=128)[c])
                cont.append(t)
            wup_v = wup_sb[:].rearrange("p (c y m) -> p c y m", c=4, y=3)

            # ---- dyn convs: c1stack -> c2stack -> c3stack ----------------
            def dyn_conv(src_stack, dst_stack, F, func, scale, eng_repl):
                fv = filt[F][:].rearrange("p (y o) -> p y o", y=3)
                for (r0, nr) in ROW_TILES:
                    N = nr * PW
                    col0 = GUARD + (r0 + 1) * PW
                    pst = dyn_psp.tile([INNER, 7 * PW], F32, tag="dyn_ps")
                    ps = pst[:, 0:N]
                    nc.tensor.matmul(
                        ps, fv[:, 0:2, :],
                        _ky_pair_ap(src_stack[:], col0 - PW, N),
                        start=True, stop=False, perf_mode=DR)
                    nc.tensor.matmul(
                        ps, fv[:, 2, :],
                        src_stack[:][0:96, col0 + PW:col0 + PW + N],
                        start=False, stop=True)
                    nc.scalar.activation(
                        _interior(dst_stack[32:64, col0:col0 + N], nr),
                        _interior(ps, nr),
                        func, scale=scale, alpha=0.2)
                    if eng_repl:
                        # per-tile kx replication via idle engines (keeps the
                        # next conv's pipeline fine-grained, no DMA boundary)
                        row = dst_stack[32:64, col0:col0 + N].rearrange(
                            "p (r x) -> p r x", x=PW)
                        b0 = dst_stack[0:32, col0:col0 + N].rearrange(
                            "p (r x) -> p r x", x=PW)
                        b2 = dst_stack[64:96, col0:col0 + N].rearrange(
                            "p (r x) -> p r x", x=PW)
                        nc.gpsimd.tensor_copy(b0[:, :, 1:PW], row[:, :, 0:PW - 1])
                        nc.vector.tensor_copy(b2[:, :, 0:PW - 1], row[:, :, 1:PW])
                    elif r0 in (21, 42, 63):
                        _replicate(nc, dst_stack, r0)

            dyn_conv(c1stack, c2stack, 0, Lrelu, S_C2 / S_F, True)
            dyn_conv(c2stack, c3stack, 1, Identity, S_C3 / (S_C2 * S_F), False)

            # ---- up conv + residual -------------------------------------
            # tiles processed in stationary-sharing pairs: MM1(t), MM1(t+1),
            # MM2(t), MM2(t+1) so LDWEIGHTS is reused across two tiles
            inv_up = 1.0 / (S_UPW * S_C3)

            def up_mm1(cc, r0, nr):
                pst = up_psp.tile([128, 7 * PW], F32, tag="up_ps")
                ps = pst[:, 0:nr * PW]
                col0 = GUARD + (r0 + 1) * PW
                nc.tensor.matmul(
                    ps, wup_v[0:96, cc, 0:2, :],
                    _ky_pair_ap(c3stack[:], col0 - PW, nr * PW),
                    start=True, stop=False, perf_mode=DR)
                return ps

            def up_mm2_post(cc, r0, nr, ps):
                col0 = GUARD + (r0 + 1) * PW
                nc.tensor.matmul(
                    ps, wup_v[:, cc, 2, :],
                    c3stack[:][0:97, col0 + PW:col0 + PW + nr * PW],
                    start=False, stop=True)
                dst = cont[cc][:, r0 * W:(r0 + nr) * W].rearrange(
                    "p (r x) -> p r x", x=W)
                if cc != 1:
                    nc.vector.scalar_tensor_tensor(
                        dst, _interior(ps, nr), inv_up, dst,
                        op0=AluMult, op1=AluAdd)
                else:
                    # spread one chunk's post over ACT + gpsimd
                    tmt = work.tile([128, 7 * W], BF16, tag="uptmp")
                    tm = tmt[:, 0:nr * W]
                    nc.scalar.activation(
                        tm.rearrange("p (r x) -> p r x", x=W),
                        _interior(ps, nr),
                        mybir.ActivationFunctionType.Copy, scale=inv_up)
                    flat = cont[cc][:, r0 * W:(r0 + nr) * W]
                    nc.gpsimd.tensor_add(flat, flat, tm)

            for cc in range(4):
                for ti in range(0, 10, 2):
                    r0a, nra = ROW_TILES[ti]
                    r0b, nrb = ROW_TILES[ti + 1]
                    psa = up_mm1(cc, r0a, nra)
                    psb = up_mm1(cc, r0b, nrb)
                    up_mm2_post(cc, r0a, nra, psa)
                    up_mm2_post(cc, r0b, nrb, psb)
                    if r0b == 35:
                        nc.sync.dma_start(
                            out=out_h[:].rearrange("(c p) q -> c p q", p=128)[cc]
                                [:, 0:42 * W],
                            in_=cont[cc][:, 0:42 * W])
                nc.sync.dma_start(
                    out=out_h[:].rearrange("(c p) q -> c p q", p=128)[cc]
                        [:, 42 * W:],
                    in_=cont[cc][:, 42 * W:])

    nc.compile()
    return nc


_NC_CACHE = None


def _get_nc():
    global _NC_CACHE
    if _NC_CACHE is None:
        _NC_CACHE = _build_program()
    return _NC_CACHE


def _to_fp8(x):
    return np.clip(x, -240.0, 240.0).astype(NP_FP8)


def _pad_image_fp8(img):
    """img [128, 64, 64] f32 -> [BUFW] padded+guarded fp8 row-block."""
    out = np.zeros((128, BUFW), np.float32)
    pad = out[:, GUARD:GUARD + NPAD].reshape(128, H + 2, PW)
    pad[:, 1:1 + H, 1:1 + W] = img
    return _to_fp8(out)


def _prep_static(ds_w, up_w, up_b, f1_cw, f2_cw, f1_fw, f2_fw, f1_fb, f2_fb):
    # w_ds [128, 6, 2, 96]: piece e = ky*2 + pair; value = S_DSW *
    #   ds_w[o, pair*256 + j*128 + k, ky, kx] at free col (kx*32 + o)
    wds = np.zeros((128, 6, 2, 96), np.float32)
    for ky in range(3):
        for pair in range(2):
            for j in range(2):
                blk = ds_w[:, pair * 256 + j * 128: pair * 256 + (j + 1) * 128, ky, :]
                # blk [o, k, kx] -> [k, (kx, o)]
                wds[:, ky * 2 + pair, j, :] = S_DSW * blk.transpose(1, 2, 0).reshape(128, 96)
    # w_cw [128, 4, 9, 64]: value = cwF[o, c*128+k, t]; cols (F*32 + o)
    wcw = np.zeros((128, 4, 9, 64), np.float32)
    for c in range(4):
        for F, cw in enumerate((f1_cw, f2_cw)):
            blk = cw[:, c * 128:(c + 1) * 128, :, :].reshape(32, 128, 9)
            wcw[:, c, :, F * 32:(F + 1) * 32] = blk.transpose(1, 2, 0)
    # w_fc [128, 2, 24, 96]: lhsT[(g4*32 + kk), F, g, (kx*32+i)] =
    #   S_F * fwF[o*288 + i*9 + ky*3 + kx, kk] with (ky,o) = divmod(g*4+g4, 32)
    wfc = np.zeros((128, 2, 24, 96), np.float32)
    for F, fw in enumerate((f1_fw, f2_fw)):
        fw4 = fw.reshape(32, 32, 3, 3, 32)  # [o, i, ky, kx, kk]
        for g in range(24):
            for g4 in range(4):
                ky, o = divmod(g * 4 + g4, 32)
                # [i, kx, kk] -> [kk, (kx, i)]
                blk = fw4[o, :, ky, :, :]
                wfc[g4 * 32:(g4 + 1) * 32, F, g, :] = (
                    S_F * blk.transpose(2, 1, 0).reshape(32, 96))
    # fb_l [96, 2, 96]: [(kx*32+i), F, (ky*32+o)] = S_F * fb[o*288+i*9+ky*3+kx]
    fbl = np.zeros((96, 2, 96), np.float32)
    for F, fb in enumerate((f1_fb, f2_fb)):
        fb4 = fb.reshape(32, 32, 3, 3)  # [o, i, ky, kx]
        fbl[:, F, :] = S_F * np.transpose(fb4, (3, 1, 2, 0)).reshape(96, 96)
    # w_up [97, 4, 3, 128]: rows (kx*32+i) = S_UPW*up_w[cc*128+o', i, ky, kx];
    #   row 96 = (S_UPW*S_C3/BIAS_C0)*up_b[cc*128+o'] on the ky=2 piece
    wup = np.zeros((97, 4, 3, 128), np.float32)
    for cc in range(4):
        blk = up_w[cc * 128:(cc + 1) * 128, :, :, :]  # [o', i, ky, kx]
        wup[0:96, cc, :, :] = S_UPW * blk.transpose(3, 1, 2, 0).reshape(96, 3, 128)
        wup[96, cc, 2, :] = (S_UPW * S_C3 / BIAS_C0) * up_b[cc * 128:(cc + 1) * 128]
    return (
        _to_fp8(wds.reshape(128, -1)),
        np.ascontiguousarray(wcw.reshape(128, -1)).astype(NP_BF16),
        np.ascontiguousarray(wfc.reshape(128, -1)).astype(NP_BF16),
        np.ascontiguousarray(fbl.reshape(96, -1)).astype(np.float32),
        _to_fp8(wup.reshape(97, -1)),
    )


def _rect_mask():
    """mask [128, 32, 16] bf16: rect membership per pixel, 9 taps + 7 pad."""
    y = (np.arange(NPIX) // W).reshape(32, 128)
    x = (np.arange(NPIX) % W).reshape(32, 128)
    m = np.zeros((128, 32, 16), np.float32)
    for t in range(9):
        ky, kx = divmod(t, 3)
        ok = np.ones((32, 128), bool)
        if ky == 0:
            ok &= y != H - 1
        elif ky == 2:
            ok &= y != 0
        if kx == 0:
            ok &= x != W - 1
        elif kx == 2:
            ok &= x != 0
        m[:, :, t] = ok.T
    return _to_fp8(np.ascontiguousarray(m.reshape(128, -1)))


def kernel(content, style, ds_w, ds_b, up_w, up_b,
           f1_cw, f1_cb, f1_fw, f1_fb,
           f2_cw, f2_cb, f2_fw, f2_fb):
    content = np.asarray(content, np.float32)
    style = np.asarray(style, np.float32)
    B = content.shape[0]
    assert B == 8

    wds, wcw, wfc, fbl, wup = _prep_static(
        np.asarray(ds_w, np.float32), np.asarray(up_w, np.float32),
        np.asarray(up_b, np.float32),
        np.asarray(f1_cw, np.float32), np.asarray(f2_cw, np.float32),
        np.asarray(f1_fw, np.float32), np.asarray(f2_fw, np.float32),
        np.asarray(f1_fb, np.float32), np.asarray(f2_fb, np.float32))
    cbb = np.concatenate([np.asarray(f1_cb, np.float32),
                          np.asarray(f2_cb, np.float32)])

    shared = {
        "w_ds": wds, "w_cw": wcw, "w_fc": wfc, "fb_l": fbl, "w_up": wup,
        "ds_b": np.asarray(ds_b, np.float32), "cb_b": cbb,
        "mask": _rect_mask(),
        "eye16": np.eye(16, dtype=np.float32).astype(NP_BF16),
    }
    in_maps = []
    for b in range(B):
        m = dict(shared)
        cimg = content[b].reshape(4, 128, H, W)
        m["cpad0"] = np.concatenate(
            [_pad_image_fp8(cimg[0]), _pad_image_fp8(cimg[1])], axis=1)
        m["cpad1"] = np.concatenate(
            [_pad_image_fp8(cimg[2]), _pad_image_fp8(cimg[3])], axis=1)
        m["content_bf"] = np.ascontiguousarray(
            content[b].reshape(CIN, NPIX)).astype(NP_BF16)
        # styleT [p, k, ch] = style[ch, k*128 + p]
        m["styleT"] = _to_fp8(np.ascontiguousarray(
            style[b].reshape(CIN, 32, 128).transpose(2, 1, 0)
            .reshape(128, 32 * CIN)))
        in_maps.append(m)

    nc = _get_nc()
    trace = bool(int(os.environ.get("KF_TRACE", "0")))
    res = run_bass_kernel_spmd(nc, in_maps, core_ids=list(range(B)), trace=trace)
    if trace and getattr(res, "exec_time_ns", None) is not None:
        print(f"HW exec time: {res.exec_time_ns} ns")
        kernel.last_exec_ns = res.exec_time_ns
    kernel.last_results = res
    out = np.stack([res.results[b]["out"].astype(np.float32).reshape(CIN, H, W)
                    for b in range(B)])
    return out


if __name__ == "__main__":
    _get_nc()
    print("program built + compiled OK")
